# revision 24
# baseline (speedup 1.0000x reference)
"""Trainium2 Bass kernel for LinearAttention4 (self-contained).

Problem (per sample): x [256, 56, 56] fp32
  qk = elu(conv1x1(x; qk_w, qk_b)) + 1 ; q, k = split(qk)
  kv = k @ v.T / n ; num = q.T @ kv ; den = q.T @ mean(k) + 1e-6
  attn = (num / den).T ; out = attn + depthwise3x3(x; pe_w) + pe_b

Sharding: data-parallel over batch, 4 samples per core on 8 NeuronCores.

v2 design (vs v1): x streams in bf16 (halves input DMA, 1 cyc/row matmuls
at any width); the x^T / k^T operands needed by the kv contraction are
produced by XBAR DMA-transposes (14 ns/16x128 tile, bf16-only) instead of
87 PE transposes + 29 PSUM->SBUF evacuation copies per sample; the 3x3
depthwise conv runs as 9 diagonal matmuls per channel block of which 5
accumulate in PSUM on the PE (with the attention numerator) and 2+2 are
applied post-evacuation on DVE / Pool via scalar_tensor_tensor with
per-partition weight scalars. elu(z)+1 = min(exp(z+b),1) + relu(z+b) with
exp and relu on ACT (one shared activation table), combine on DVE.

Spatial layout: zero-padded 58x58 grid (NP=3364) in a 3456-wide buffer
(27*128, XBAR-aligned). Conv taps are +-{58,1} offsets; pad columns yield
garbage skipped by the evacuation AP. kv contracts over all padded
positions: x pads are zero so they contribute nothing.
"""

import numpy as np

import concourse.bass as bass
import concourse.mybir as mybir
from concourse.tile import TileContext
from concourse.bass_utils import run_bass_kernel_spmd

F32 = mybir.dt.float32
F32R = mybir.dt.float32r
BF16 = mybir.dt.bfloat16

B, C, H, W = 32, 256, 56, 56
N = H * W  # 3136
NCORES = 8
SPC = B // NCORES  # 4
HP = H + 2  # 58
NP = HP * HP  # 3364
NPP = 27 * 128  # 3456 padded free width (XBAR tile aligned)
NTC = NPP // 128  # 27 transpose chunks
SPAN = 8 * HP  # 464 cols per qk/num/conv chunk (8 padded rows)
NCH = 7  # chunks of 8 interior rows
EPS = 1e-6 * N  # den eps, rescaled because kv/k_sum stay unscaled

PE_TAPS = (0, 1, 2, 3, 4, 5, 6)
DVE_TAPS = (7, 8)
POOL_TAPS = ()


def _split_multi_waits(nc, max_waits=1):
    """Walrus here allows one SyncWait per instruction; hoist extras onto
    fresh same-engine NOPs placed immediately before (same semantics)."""
    for f in nc.m.functions:
        for blk in f.blocks:
            new_insts = []
            for ins in blk.instructions:
                si = ins.sync_info
                waits = list(si.on_wait) if si is not None else []
                if len(waits) > max_waits:
                    head, tail = waits[:-max_waits], waits[-max_waits:]
                    for w in head:
                        nop = mybir.InstNoOp(
                            name=f"Wsplit-{nc.next_id()}", engine=ins.engine,
                            ins=[], outs=[],
                        )
                        nop.sync_info = mybir.SyncInfo(on_wait=[w], on_update=[])
                        new_insts.append(nop)
                    ins.sync_info = mybir.SyncInfo(
                        on_wait=tail, on_update=list(si.on_update)
                    )
                new_insts.append(ins)
            blk.instructions = new_insts


def _build():
    nc = bass.Bass()
    # all DRAM params are flat 1D: PJRT/XLA may permute multi-dim parameter
    # layouts; 1D is unambiguous
    xs_f = nc.declare_dram_parameter("xs", [SPC * 2 * 128 * NPP], BF16, isOutput=False)
    wqkT_f = nc.declare_dram_parameter("wqkT", [2 * 128 * 256], BF16, isOutput=False)
    wtap_f = nc.declare_dram_parameter("wtap", [2 * 9 * 128 * 128], BF16, isOutput=False)
    wtpp_f = nc.declare_dram_parameter("wtpp", [128 * 18], F32, isOutput=False)
    ones_f = nc.declare_dram_parameter("ones", [128 * 128], BF16, isOutput=False)
    biasqk_f = nc.declare_dram_parameter("biasqk", [128 * 2], F32, isOutput=False)
    peb_f = nc.declare_dram_parameter("peb", [128 * 2], F32, isOutput=False)
    out_f = nc.declare_dram_parameter("out", [SPC * 2 * 128 * N], BF16, isOutput=True)
    xs = xs_f[:].rearrange("(s c p n) -> s p c n", s=SPC, c=2, p=128)
    out = out_f[:].rearrange("(s c p n) -> s c p n", s=SPC, c=2, p=128)

    Exp = mybir.ActivationFunctionType.Exp
    Relu = mybir.ActivationFunctionType.Relu
    Ident = mybir.ActivationFunctionType.Identity
    mi, ad, mx = mybir.AluOpType.min, mybir.AluOpType.add, mybir.AluOpType.max
    mu = mybir.AluOpType.mult

    def span_start(ch):
        # first output position of chunk ch, in padded coords
        return HP * (1 + 8 * ch) + 1

    with TileContext(nc) as tc:
        with (
            tc.tile_pool(name="wp", bufs=1) as wp,
            tc.tile_pool(name="xpool", bufs=2) as xpool,
            tc.tile_pool(name="qkpool", bufs=3) as qkpool,
            tc.tile_pool(name="erpool", bufs=4) as erpool,
            tc.tile_pool(name="tpool", bufs=3) as tpool,
            tc.tile_pool(name="kvpool", bufs=2) as kvpool,
            tc.tile_pool(name="denpool", bufs=2) as denpool,
            tc.tile_pool(name="opool", bufs=4) as opool,
            tc.tile_pool(name="bigps", bufs=3, space="PSUM") as bigps,
            tc.tile_pool(name="kvps", bufs=2, space="PSUM") as kvps,
            tc.tile_pool(name="dbps", bufs=3, space="PSUM") as dbps,
        ):
            w_qk = wp.tile([128, 512], BF16, name="w_qk")
            w_tap = wp.tile([128, 2304], BF16, name="w_tap")
            w_tpp = wp.tile([128, 18], F32, name="w_tpp")
            w_ones = wp.tile([128, 128], BF16, name="w_ones")
            w_bqk = wp.tile([128, 2], F32, name="w_bqk")
            w_peb = wp.tile([128, 2], F32, name="w_peb")
            dma = nc.default_dma_engine.dma_start
            dma(
                out=w_qk[:].rearrange("p (c o) -> p c o", c=2),
                in_=wqkT_f[:].rearrange("(c p o) -> p c o", c=2, p=128),
            )
            dma(
                out=w_tap[:].rearrange("p (c t j) -> p c t j", c=2, t=9),
                in_=wtap_f[:].rearrange("(c t p j) -> p c t j", c=2, t=9, p=128),
            )
            dma(out=w_tpp[:], in_=wtpp_f[:].rearrange("(p j) -> p j", p=128))
            dma(out=w_ones[:], in_=ones_f[:].rearrange("(p j) -> p j", p=128))
            dma(out=w_bqk[:], in_=biasqk_f[:].rearrange("(p c) -> p c", p=128))
            dma(out=w_peb[:], in_=peb_f[:].rearrange("(p c) -> p c", p=128))

            for s in range(SPC):
                # ---- A: load padded bf16 x, start x transposes ------------
                xp = xpool.tile([128, 2 * NPP], BF16, tag="xp", name="xp")
                dma(out=xp[:].rearrange("p (c n) -> p c n", c=2), in_=xs[s])
                xT0 = tpool.tile([128, NPP], BF16, tag="xT0", name="xT0")
                xT1 = tpool.tile([128, NPP], BF16, tag="xT1", name="xT1")
                nc.default_dma_engine.dma_start_transpose(
                    xT0[:].rearrange("p (t c) -> p t c", c=128), xp[:, 0:NPP]
                )
                nc.default_dma_engine.dma_start_transpose(
                    xT1[:].rearrange("p (t c) -> p t c", c=128), xp[:, NPP : 2 * NPP]
                )

                # ---- B: qk matmul + elu+1 ---------------------------------
                q_elu = qkpool.tile([128, NP], BF16, tag="qelu", name="q_elu")
                k_elu = qkpool.tile([128, NPP], BF16, tag="kelu", name="k_elu")
                ksum7 = denpool.tile([128, 8], F32, tag="ksum7", name="ksum7")
                ksum = denpool.tile([128, 2], BF16, tag="ksum", name="ksum")
                # zero k's pad positions at tile birth (elu writes interior
                # only) so the kv/k_sum contraction over all padded positions
                # matches the dense reference exactly
                nc.gpsimd.memset(k_elu[:, 0:59], 0)
                nc.gpsimd.memset(k_elu[:, NP - 58 : NPP], 0)
                nc.gpsimd.memset(
                    k_elu[:, 0:NP].rearrange("p (y x) -> p y x", y=HP)[:, 1:57, 0:1], 0
                )
                nc.gpsimd.memset(
                    k_elu[:, 0:NP].rearrange("p (y x) -> p y x", y=HP)[:, 1:57, 57:58],
                    0,
                )
                nc.vector.memset(ksum7[:].bitcast(F32), 0)
                for mb in range(2):  # 0 = q, 1 = k
                    for ch in range(NCH):
                        p1 = span_start(ch)
                        ps = bigps.tile([128, SPAN], F32, tag="bigps", name="ps")
                        for cc in range(2):
                            nc.tensor.matmul(
                                ps[:],
                                w_qk[:, 256 * cc + 128 * mb : 256 * cc + 128 * mb + 128],
                                xp[:, NPP * cc + p1 : NPP * cc + p1 + SPAN],
                                start=(cc == 0),
                                stop=(cc == 1),
                            )
                        e = erpool.tile([128, SPAN], F32, tag="e", name="e")
                        r = erpool.tile([128, SPAN], F32, tag="r", name="r")
                        nc.scalar.activation(
                            e[:], ps[:], Exp, bias=w_bqk[:, mb : mb + 1], scale=1.0
                        )
                        nc.scalar.activation(
                            r[:], ps[:], Relu, bias=w_bqk[:, mb : mb + 1], scale=1.0
                        )
                        e_v = e[:].rearrange("p (a b) -> p a b", b=HP)[:, :, 0:56]
                        r_v = r[:].rearrange("p (a b) -> p a b", b=HP)[:, :, 0:56]
                        if mb == 0:
                            dst_v = q_elu[:, p1 : p1 + SPAN].rearrange(
                                "p (a b) -> p a b", b=HP
                            )[:, :, 0:56]
                            nc.vector.scalar_tensor_tensor(
                                dst_v, e_v, 1.0, r_v, op0=mi, op1=ad
                            )
                        else:
                            dst_v = k_elu[:, p1 : p1 + SPAN].rearrange(
                                "p (a b) -> p a b", b=HP
                            )[:, :, 0:56]
                            nc.vector.scalar_tensor_tensor(
                                dst_v, e_v, 1.0, r_v, op0=mi, op1=ad,
                                accum_out=ksum7[:, ch : ch + 1],
                            )
                with nc.allow_low_precision(
                    reason="ksum reduce to f32r: feeds f32r matmul anyway"
                ):
                    nc.vector.tensor_reduce(
                        ksum[:, 0:1], ksum7[:], op=mybir.AluOpType.add,
                        axis=mybir.AxisListType.X,
                    )

                # ---- C: k transpose + kv ----------------------------------
                kT = tpool.tile([128, NPP], BF16, tag="kT", name="kT")
                nc.default_dma_engine.dma_start_transpose(
                    kT[:].rearrange("p (t c) -> p t c", c=128), k_elu[:]
                )
                kvp = kvps.tile([128, 256], F32, tag="kvps", name="kvp")
                kv_sb = kvpool.tile([128, 256], BF16, tag="kv", name="kv_sb")
                for j in range(NTC):
                    nc.tensor.matmul(
                        kvp[:, 0:128], kT[:, 128 * j : 128 * (j + 1)],
                        xT0[:, 128 * j : 128 * (j + 1)],
                        start=(j == 0), stop=(j == NTC - 1),
                    )
                    nc.tensor.matmul(
                        kvp[:, 128:256], kT[:, 128 * j : 128 * (j + 1)],
                        xT1[:, 128 * j : 128 * (j + 1)],
                        start=(j == 0), stop=(j == NTC - 1),
                    )
                nc.scalar.copy(kv_sb[:], kvp[:])

                # ---- D: den + reciprocal + q scaling ----------------------
                den = denpool.tile([1, NP], BF16, tag="den", name="den")
                for ch in range(NCH):
                    p1 = span_start(ch)
                    dp = dbps.tile([128, SPAN], F32, tag="dbps", name="dp")
                    nc.tensor.matmul(
                        dp[0:1, :], ksum[:, 0:1],
                        q_elu[:, p1 : p1 + SPAN],
                        start=True, stop=True,
                    )
                    nc.scalar.copy(den[:, p1 : p1 + SPAN], dp[0:1, :])
                recf = denpool.tile([116, 29], BF16, tag="recf", name="recf")
                dma(out=recf[:], in_=den[:])
                nc.vector.tensor_scalar_add(recf[:], recf[:], EPS)
                with nc.allow_low_precision(
                    reason="bf16 reciprocal: den ~O(n), rel err fine"
                ):
                    nc.vector.reciprocal(recf[:], recf[:])
                rrow = denpool.tile([1, NP], BF16, tag="rrow", name="rrow")
                dma(out=rrow[:], in_=recf[:])
                for ch in range(NCH):
                    p1 = span_start(ch)
                    bc = dbps.tile([128, SPAN], F32, tag="dbps", name="bc")
                    nc.tensor.matmul(
                        bc[:], w_ones[0:1, :], rrow[:, p1 : p1 + SPAN],
                        start=True, stop=True,
                    )
                    nc.vector.tensor_mul(
                        q_elu[:, p1 : p1 + SPAN], q_elu[:, p1 : p1 + SPAN], bc[:]
                    )

                # ---- E: num + PE taps into psum; evac; DVE/Pool taps ------
                for cb in range(2):
                    o_sb = opool.tile([128, N], BF16, tag="osb", name="o_sb")
                    for ch in range(NCH):
                        p1 = span_start(ch)
                        pn = bigps.tile([128, SPAN], F32, tag="bigps", name="pn")
                        nc.tensor.matmul(
                            pn[:], kv_sb[:, 128 * cb : 128 * (cb + 1)],
                            q_elu[:, p1 : p1 + SPAN],
                            start=True, stop=False,
                        )
                        pe_taps = PE_TAPS if cb else PE_TAPS[:-1]
                        for i, t in enumerate(pe_taps):
                            ky, kx = t // 3, t % 3
                            off = HP * (ky - 1) + (kx - 1)
                            nc.tensor.matmul(
                                pn[:],
                                w_tap[:, 1152 * cb + 128 * t : 1152 * cb + 128 * (t + 1)],
                                xp[:, NPP * cb + p1 + off : NPP * cb + p1 + off + SPAN],
                                start=False, stop=(i == len(pe_taps) - 1),
                            )
                        o_v = o_sb[:, 448 * ch : 448 * (ch + 1)].rearrange(
                            "p (y x) -> p y x", x=56
                        )
                        nc.scalar.activation(
                            o_v,
                            pn[:].rearrange("p (y x) -> p y x", x=HP)[:, :, 0:56],
                            Ident, bias=w_peb[:, cb : cb + 1], scale=1.0,
                        )
                        dve_taps = DVE_TAPS if cb else DVE_TAPS + (PE_TAPS[-1],)
                        for t in dve_taps:
                            ky, kx = t // 3, t % 3
                            off = HP * (ky - 1) + (kx - 1)
                            x_v = xp[
                                :, NPP * cb + p1 + off : NPP * cb + p1 + off + SPAN
                            ].rearrange("p (a b) -> p a b", b=HP)[:, :, 0:56]
                            nc.vector.scalar_tensor_tensor(
                                o_v, x_v, w_tpp[:, 9 * cb + t : 9 * cb + t + 1],
                                o_v, op0=mu, op1=ad,
                            )
                        for t in POOL_TAPS:
                            ky, kx = t // 3, t % 3
                            off = HP * (ky - 1) + (kx - 1)
                            x_v = xp[
                                :, NPP * cb + p1 + off : NPP * cb + p1 + off + SPAN
                            ].rearrange("p (a b) -> p a b", b=HP)[:, :, 0:56]
                            nc.gpsimd.scalar_tensor_tensor(
                                o_v, x_v, w_tpp[:, 9 * cb + t : 9 * cb + t + 1],
                                o_v, op0=mu, op1=ad,
                            )
                    dma(out=out[s, cb], in_=o_sb[:])

    _split_multi_waits(nc)
    return nc


_NC_CACHE = []


def kernel(x, qk_w, qk_b, pe_w, pe_b):
    import ml_dtypes

    x = np.asarray(x, np.float32)
    qk_w = np.asarray(qk_w, np.float32)
    qk_b = np.asarray(qk_b, np.float32)
    pe_w = np.asarray(pe_w, np.float32)
    pe_b = np.asarray(pe_b, np.float32)

    # host prep: zero-padded 58x58 spatial layout (3456-wide, XBAR aligned),
    # c in two partition blocks, bf16
    xp = np.zeros((B, 2, 128, NPP), ml_dtypes.bfloat16)
    xg = xp[:, :, :, : NP].reshape(B, 2, 128, HP, HP)
    xg[:, :, :, 1 : H + 1, 1 : W + 1] = x.reshape(B, 2, 128, H, W)

    wqkT = np.ascontiguousarray(qk_w.T).reshape(2, 128, 256).astype(ml_dtypes.bfloat16)
    wtap = np.zeros((2, 9, 128, 128), ml_dtypes.bfloat16)
    wtpp = np.zeros((128, 18), np.float32)
    idx = np.arange(128)
    for cb in range(2):
        for t in range(9):
            wv = pe_w[128 * cb : 128 * (cb + 1), 0, t // 3, t % 3]
            wtap[cb, t, idx, idx] = wv
            wtpp[:, 9 * cb + t] = wv
    biasqk = np.stack([qk_b[:128], qk_b[128:]], axis=1).copy()  # [128, 2]
    pebh = np.stack([pe_b[:128], pe_b[128:]], axis=1).copy()

    shared = {
        "wqkT": wqkT.ravel(),
        "wtap": wtap.ravel(),
        "wtpp": wtpp.ravel(),
        "ones": np.ones(128 * 128, ml_dtypes.bfloat16),
        "biasqk": biasqk.ravel(),
        "peb": pebh.ravel(),
    }
    in_maps = [
        {"xs": np.ascontiguousarray(xp[r * SPC : (r + 1) * SPC]).ravel(), **shared}
        for r in range(NCORES)
    ]

    if not _NC_CACHE:
        _NC_CACHE.append(_build())
    nc = _NC_CACHE[0]
    res = run_bass_kernel_spmd(nc, in_maps, list(range(NCORES)))

    full = np.empty((B, C, H, W), np.float32)
    for r in range(NCORES):
        o = res.results[r]["out"].reshape(SPC, 2, 128, N).astype(np.float32)
        full[r * SPC : (r + 1) * SPC] = o.reshape(SPC, C, H, W)
    return full
